# revision 5
# baseline (speedup 1.0000x reference)
"""Block-sparse DSD matmul  y = x @ W^T  on 8 TRN2 NeuronCores.

x: [2048, 4096] f32, W given as 2048 sparse 32x32 blocks at (rows, cols)
block coordinates in a 128x128 block grid. y: [2048, 4096] f32.

Strategy (batch-parallel SPMD, identical program on 8 cores):
  - Shard batch 8 ways (256 rows/core); the sparse structure is identical
    on every core so one SPMD program works with per-core x shards.
  - All tensors cast to bf16 on host: PE matmuls run 1-pass (fp32 ran
    LOW_HIGH 2-pass) and HBM traffic halves.  PSUM accumulates f32;
    y is written back bf16 and widened on host.
  - Compute y^T tiles on-chip: for block (r, c):
        y^T[32r:32r+32, :] += W_blk @ x^T[32c:32c+32, :]
    As a PE matmul: out = lhsT.T @ rhs with lhsT = W_blk^T (stationary,
    32x32), rhs = x^T chunk [32, 256].
  - 16-way 32x32 PE subarray tiling: lane a = c%4 picks the SBUF
    partition strip (and PE row group); row-blocks are packed 4 to a
    "group", strip b in the group picks the PSUM partition strip (PE col
    group).  Each lane accumulates into its own PSUM bank; the 4 lane
    banks fold via ACT (2 copies), DVE (2 adds) and Pool (final add +
    bf16 cast) so no single engine serializes the evacuation.
  - Weights are packed PER LANE (each 32-partition strip has its own
    column layout, no group-alignment padding) and slots are emitted in
    near-perfect b round-robin so a subarray is revisited only every
    ~16 slots (the PE serializes LDWEIGHTS against the same subarray's
    in-flight matmul).
  - DMA: x in 5 chunks + w in 9 chunks on the Sync HWDGE queue, first
    chunks small so the PE starts ~2 us in; y^T written per 4 groups on
    the Scalar HWDGE queue so output never queues behind input loads.
  - Host: pre-transposes x into partition-major per-core layout, packs
    transposed weight blocks into a lane-major array, assembles y.
"""

import numpy as np
import ml_dtypes

BF16 = ml_dtypes.bfloat16

# toggles used by test.py only; harness uses defaults
_RUN = {"trace": False, "trace_cores": [0], "last": None}

B, K, OUT, BLK, NNZ = 2048, 4096, 4096, 32, 2048
NCORES = 8
BC = B // NCORES          # 256 batch rows per core
NT = K // 128             # 32 x^T partition-tiles
NRB = OUT // BLK          # 128 row blocks
NG = NRB // 4             # 32 groups of 4 row blocks

# group index below which slot order stays t-monotonic (x still streaming in)
TSORT_G = 12
# w chunk boundaries (in groups) and x chunk boundaries (in tiles):
# small leading chunks prime the pipeline.
WCHUNKS = [1, 3] + [4] * 7
XCHUNKS = [2, 6, 8, 8, 8]


def _build_schedule(w, rows, cols):
    """Group assignment + per-(group, lane) slot schedule + packed weights.

    Returns (prog, lane_off, lane_len, wpk, rmap) where prog[g][a] is a
    list of slots (t, b, start, stop), lane_off[g][a] is the slot column
    offset of group g in lane a's strip, and wpk[128, max_lane_len*32]
    holds W^T blocks packed per lane strip.
    """
    cnt = np.bincount(rows, minlength=NRB)
    order = np.argsort(-cnt, kind="stable")
    rmap = np.empty((NG, 4), dtype=np.int64)
    for rank, r in enumerate(order):
        rnd, pos = rank // NG, rank % NG
        g = pos if rnd % 2 == 0 else NG - 1 - pos
        rmap[g, rnd] = r

    gb_of_row = {}
    for g in range(NG):
        for b in range(4):
            gb_of_row[int(rmap[g, b])] = (g, b)

    cells = [[[[] for _ in range(4)] for _ in range(4)] for _ in range(NG)]
    for n in range(NNZ):
        g, b = gb_of_row[int(rows[n])]
        cells[g][int(cols[n]) % 4][b].append(n)

    prog = []
    wts = [[] for _ in range(4)]   # per-lane packed W^T blocks, in slot order
    lane_off = []
    for g in range(NG):
        lanes = []
        offs_g = []
        for a in range(4):
            raw = []
            for b in range(4):
                cl = cells[g][a][b]
                if not cl:
                    raw.append((0, b, np.zeros((BLK, BLK), np.float32)))
                for n in cl:
                    raw.append((int(cols[n]) // 4, b,
                                np.ascontiguousarray(w[n].T)))
            if g < TSORT_G:
                # x still streaming: keep t-monotonic, greedily avoid
                # repeating b within a small t window
                raw.sort(key=lambda s: s[0])
                reordered = []
                pend = list(raw)
                recent = []
                while pend:
                    pick = 0
                    best = -1
                    for j in range(min(8, len(pend))):
                        if pend[j][0] > pend[0][0] + 2:
                            break
                        d = (len(recent) - recent[::-1].index(pend[j][1])
                             if pend[j][1] in recent else 99)
                        if d > best:
                            best = d
                            pick = j
                        if d == 99:
                            break
                    slot = pend.pop(pick)
                    reordered.append(slot)
                    recent.append(slot[1])
                    if len(recent) > 3:
                        recent.pop(0)
                raw = reordered
            else:
                # x resident: ignore t, emit a perfect b round-robin
                byb = [[s for s in raw if s[1] == b] for b in range(4)]
                for lst in byb:
                    lst.sort(key=lambda s: s[0])
                total = len(raw)
                # rate-balanced merge: each b-class spread evenly
                sched = []
                idxf = [0.0] * 4
                done = [0] * 4
                for _ in range(total):
                    best_b, best_v = -1, 1e9
                    for b in range(4):
                        if done[b] < len(byb[b]):
                            v = (done[b] + 0.5) / len(byb[b])
                            if v < best_v:
                                best_v, best_b = v, b
                    sched.append(byb[best_b][done[best_b]])
                    done[best_b] += 1
                raw = sched
            first = {}
            last = {}
            for i, (_, b, _) in enumerate(raw):
                first.setdefault(b, i)
                last[b] = i
            slots = [(t, b, i == first[b], i == last[b])
                     for i, (t, b, _) in enumerate(raw)]
            lanes.append(slots)
            offs_g.append(len(wts[a]))
            for _, _, wt in raw:
                wts[a].append(wt)
        prog.append(lanes)
        lane_off.append(offs_g)

    lane_len = [len(wts[a]) for a in range(4)]
    tot = max(lane_len)
    wpk = np.zeros((128, tot * BLK), dtype=np.float32)
    for a in range(4):
        for idx, wt in enumerate(wts[a]):
            wpk[32 * a:32 * a + 32, idx * BLK:(idx + 1) * BLK] = wt
    return prog, lane_off, lane_len, tot, wpk, rmap


def kernel(x, w, rows, cols, out_blocks=None):
    import concourse.bass as bass
    import concourse.bacc as bacc
    import concourse.tile as tile
    import concourse.mybir as mybir
    from concourse.bass_utils import run_bass_kernel_spmd
    from contextlib import ExitStack

    x = np.asarray(x, dtype=np.float32)
    w = np.asarray(w, dtype=np.float32)
    rows = np.asarray(rows).astype(np.int64)
    cols = np.asarray(cols).astype(np.int64)

    prog, lane_off, lane_len, tot, wpk, rmap = _build_schedule(w, rows, cols)
    wpk16 = wpk.astype(BF16)

    # w chunk boundaries, in slot units per lane (chunk k covers groups
    # wg[k]..wg[k+1]; lane a's strip columns lane_off[wg[k]][a]..)
    wg = np.cumsum([0] + WCHUNKS)           # group boundaries, wg[-1] == NG
    xg = np.cumsum([0] + XCHUNKS)           # x tile boundaries, xg[-1] == NT

    # x^T, per-core partition-major: xarr[core, p, t*BC + j] = x[BC*core + j, 128*t + p]
    xarr = np.ascontiguousarray(
        x.reshape(NCORES, BC, NT, 128).transpose(0, 3, 2, 1)
    ).reshape(NCORES, 128, NT * BC).astype(BF16)

    f32 = mybir.dt.float32
    bf16 = mybir.dt.bfloat16
    nc = bacc.Bacc()
    xt_d = nc.declare_dram_parameter("xt", [128, NT * BC], bf16, isOutput=False)
    wp_d = nc.declare_dram_parameter("wpk", [128, tot * BLK], bf16, isOutput=False)
    yt_d = nc.declare_dram_parameter("yt", [128, NG * BC], bf16, isOutput=True)

    with tile.TileContext(nc) as tc, ExitStack() as ctx:
        xp = ctx.enter_context(tc.tile_pool(name="x", bufs=1))
        wpool = ctx.enter_context(tc.tile_pool(name="w", bufs=1))
        pp = ctx.enter_context(tc.tile_pool(name="ps", bufs=8, space="PSUM"))
        tp = ctx.enter_context(tc.tile_pool(name="tmp", bufs=3))
        yp = ctx.enter_context(tc.tile_pool(name="y", bufs=2))

        # Per-lane strips have different lengths per chunk; allocate each
        # chunk tile at the max lane extent and DMA the 4 strips separately.
        wtiles = {}

        def load_w(k):
            g0, g1 = int(wg[k]), int(wg[k + 1])
            exts = []
            for a in range(4):
                lo = lane_off[g0][a]
                hi = lane_off[g1][a] if g1 < NG else lane_len[a]
                exts.append((lo, hi))
            width = max(hi - lo for lo, hi in exts)
            wsb = wpool.tile([128, width * BLK], bf16, tag=f"w{k}",
                             name=f"w{k}")
            for a in range(4):
                lo, hi = exts[a]
                if hi > lo:
                    nc.sync.dma_start(
                        wsb[32 * a:32 * a + 32, :(hi - lo) * BLK],
                        wp_d[32 * a:32 * a + 32, lo * BLK:hi * BLK])
            wtiles[k] = (wsb, exts)

        xts = {}

        def load_x(ci):
            t0, t1 = int(xg[ci]), int(xg[ci + 1])
            xc = xp.tile([128, (t1 - t0) * BC], bf16, tag=f"xc{ci}",
                         name=f"xc{ci}")
            nc.sync.dma_start(xc[:], xt_d[:, t0 * BC:t1 * BC])
            for t in range(t0, t1):
                xts[t] = (xc, t - t0)

        # DMA ring is FIFO: first w chunk and first x chunk lead the queue.
        load_w(0)
        load_x(0)
        load_w(1)
        load_x(1)
        load_w(2)
        for ci in range(2, len(XCHUNKS)):
            load_x(ci)
        load_w(3)

        def rhs_of(t):
            xc, rel = xts[t]
            return xc[:, rel * BC:(rel + 1) * BC]

        nwc = len(WCHUNKS)
        kcur = 0
        y4 = None
        for g in range(NG):
            while kcur + 1 < nwc and g >= wg[kcur + 1]:
                kcur += 1
            if g == int(wg[kcur]):
                nk = kcur + 4
                if nk < nwc and nk not in wtiles:
                    load_w(nk)
            wsb, exts = wtiles[kcur]
            ps = [pp.tile([128, BC], f32, tag="ps", name=f"ps{a}")
                  for a in range(4)]
            for idx in range(max(len(prog[g][a]) for a in range(4))):
                for a in range(4):
                    if idx < len(prog[g][a]):
                        t, b, st, sp = prog[g][a][idx]
                        wcol = (lane_off[g][a] - exts[a][0] + idx) * BLK
                        nc.tensor.matmul(
                            ps[a][32 * b:32 * b + 32, :],
                            lhsT=wsb[32 * a:32 * a + 32, wcol:wcol + BLK],
                            rhs=rhs_of(t)[32 * a:32 * a + 32, :],
                            start=st, stop=sp,
                            tile_position=(32 * a, 32 * b),
                        )
            # PSUM read ports: ACT evacuates two banks, DVE folds two more
            # (one PSUM operand per DVE op), Pool does the SBUF-only final
            # add with the bf16 downcast.
            s0 = tp.tile([128, BC], f32, tag="t0")
            nc.scalar.copy(s0[:], ps[0][:])
            s2 = tp.tile([128, BC], f32, tag="t1")
            nc.scalar.copy(s2[:], ps[2][:])
            a01 = tp.tile([128, BC], f32, tag="t2")
            nc.vector.tensor_add(a01[:], s0[:], ps[1][:])
            a23 = tp.tile([128, BC], f32, tag="t3")
            nc.vector.tensor_add(a23[:], s2[:], ps[3][:])
            if g % 4 == 0:
                y4 = yp.tile([128, 4 * BC], bf16, tag="y")
            nc.gpsimd.tensor_add(
                y4[:, (g % 4) * BC:(g % 4 + 1) * BC], a01[:], a23[:])
            # flush output: every 4 groups, but split the final batch so the
            # tail DMA is smaller
            if g == NG - 2:
                nc.scalar.dma_start(
                    yt_d[:, (g - 2) * BC:(g + 1) * BC], y4[:, :3 * BC])
            elif g == NG - 1:
                nc.scalar.dma_start(
                    yt_d[:, g * BC:(g + 1) * BC],
                    y4[:, 3 * BC:4 * BC])
            elif g % 4 == 3:
                nc.scalar.dma_start(
                    yt_d[:, (g - 3) * BC:(g + 1) * BC], y4[:])

    nc.compile()

    in_maps = [{"xt": xarr[i], "wpk": wpk16} for i in range(NCORES)]
    res = run_bass_kernel_spmd(
        nc, in_maps, list(range(NCORES)),
        trace=_RUN["trace"], trace_cores=_RUN["trace_cores"],
    )
    _RUN["last"] = res

    feat = np.empty(OUT, dtype=np.int64)
    for g in range(NG):
        for b in range(4):
            feat[128 * g + 32 * b:128 * g + 32 * b + 32] = \
                32 * rmap[g, b] + np.arange(32)

    y = np.empty((B, OUT), dtype=np.float32)
    for i in range(NCORES):
        ytp = np.asarray(res.results[i]["yt"]).astype(np.float32)
        ytp = ytp.reshape(128, NG, BC).transpose(1, 0, 2).reshape(OUT, BC)
        yT = np.empty((OUT, BC), dtype=np.float32)
        yT[feat] = ytp
        y[BC * i:BC * (i + 1), :] = yT.T
    return y
